# revision 15
# baseline (speedup 1.0000x reference)
"""Trainium2 Bass kernel for nn_LocalGreedyLayer (LIF spiking layer).

Computes, for x_seq [T=16, B=512, IN=3072], fc_w [2048,3072], fc_b [2048],
aux_w [10,2048], aux_b [10]:
    cur  = x_seq @ fc_w.T + fc_b            # [T,B,OUT]
    LIF scan (tau=2, v_th=1, hard reset to 0) -> spk_seq [T,B,OUT]
    count = spk_seq.sum(0)                  # [B,OUT]
    logits = count @ aux_w.T + aux_b        # [B,10]

Sharding: data-parallel over batch B across 8 NeuronCores (64 batch rows
per core); weights replicated. Per core the matmul is computed in a
transposed layout curT [OUT, T*64] so OUT lives on SBUF partitions, the
time scan runs on [128, free] tiles (partition = out%128, free =
(out//128, b)), and spikes/counts are emitted transposed; the host
reassembles the full outputs.

Matmul modes:
  fp32   - native fp32 matmuls (4 cyc/row), bit-accurate.
  fp32r  - single-pass reduced-precision fp32 (~13 mantissa bits, 1 cyc/row).
  fp16x3 - hi/lo fp16 split: cur = xh@wh + 2^-11*(xh@wl' + xl'@wh) with
           lo parts pre-scaled by 2^11 on the host. ~22 effective mantissa
           bits (empirically fp32-equivalent spike decisions) at 3 cyc/row.
"""

import sys

sys.path.insert(0, "/opt/trn_rl_repo")

from contextlib import ExitStack

import numpy as np

import concourse.bass as bass  # noqa: F401
import concourse.tile as tile
from concourse import bacc, mybir
from concourse.bass_utils import run_bass_kernel_spmd

T, B, IN, OUT, NCLS = 16, 512, 3072, 2048, 10
NCORES = 8
BL = B // NCORES          # 64 batch rows per core
TB = T * BL               # 1024 columns of curT per core
NK = IN // 128            # 24 contraction chunks
NOC = OUT // 128          # 16 output chunks
NB = 2                    # N-blocks of 512 columns (8 timesteps each)
NBW = TB // NB            # 512
TPB = T // NB             # 8 timesteps per N-block
G = 4                     # oc groups (scan overlap granularity)
OCG = NOC // G            # 8 oc per group
GW = OCG * BL             # 512 free elements per scan-group row

F32 = mybir.dt.float32
F16 = mybir.dt.float16
BF16 = mybir.dt.bfloat16

LO_SCALE = 2048.0         # 2^11 scaling of fp16 lo parts

MM_MODE = "fp16x3"

TRACE = False
LAST_RESULTS = None


def _build_nc():
    nc = bacc.Bacc("TRN2", target_bir_lowering=False, debug=False,
                   enable_asserts=False, num_devices=NCORES)

    split = MM_MODE == "fp16x3"
    mm_dt = {"fp32": F32, "fp32r": mybir.dt.float32r, "fp16x3": F16}[MM_MODE]

    if split:
        xh_d = nc.dram_tensor("xh", [IN, TB], F16, kind="ExternalInput").ap()
        xl_d = nc.dram_tensor("xl", [IN, TB], F16, kind="ExternalInput").ap()
        wh_d = nc.dram_tensor("wh", [IN, OUT], F16, kind="ExternalInput").ap()
        wl_d = nc.dram_tensor("wl", [IN, OUT], F16, kind="ExternalInput").ap()
    else:
        xh_d = nc.dram_tensor("xh", [IN, TB], mm_dt, kind="ExternalInput").ap()
        wh_d = nc.dram_tensor("wh", [IN, OUT], mm_dt, kind="ExternalInput").ap()
    bias_d = nc.dram_tensor("bias", [128, NOC], F32, kind="ExternalInput").ap()
    auxwT = nc.dram_tensor("auxwT", [OUT, NCLS], F32, kind="ExternalInput").ap()
    auxb = nc.dram_tensor("auxb", [BL, NCLS], F32, kind="ExternalInput").ap()

    spk = nc.dram_tensor("spk", [T, NOC, 128, BL], BF16, kind="ExternalOutput").ap()
    cnt = nc.dram_tensor("cnt", [NOC, 128, BL], F32, kind="ExternalOutput").ap()
    logits = nc.dram_tensor("logits", [BL, NCLS], F32, kind="ExternalOutput").ap()

    with tile.TileContext(nc) as tc, ExitStack() as ctx:
        xres = ctx.enter_context(tc.tile_pool(name="xres", bufs=NK))
        wpool = ctx.enter_context(tc.tile_pool(name="w", bufs=3))
        curpool = ctx.enter_context(tc.tile_pool(name="cur", bufs=5))
        vpool = ctx.enter_context(tc.tile_pool(name="v", bufs=2))
        spool = ctx.enter_context(tc.tile_pool(name="s", bufs=4))
        cpool = ctx.enter_context(tc.tile_pool(name="cnt", bufs=2))
        mpool = ctx.enter_context(tc.tile_pool(name="misc", bufs=1))
        ppool = ctx.enter_context(tc.tile_pool(name="psum", bufs=4, space="PSUM"))


        wh_r0 = wh_d.rearrange("(k p) (o m) -> o p k m", p=128, m=128)
        # Preload the first oc's weight tiles ahead of the x stream so the
        # PE can start as soon as x[k=0] lands. w loads go on the sync
        # engine's DGE queues; x loads/spk stores on the scalar engine's,
        # so the two streams don't serialize behind each other.
        pre_w = {}
        wht0 = wpool.tile([128, NK * 128], mm_dt, tag="wh", name="wht0")
        nc.sync.dma_start(
            wht0[:].rearrange("p (k m) -> p k m", m=128), wh_r0[0])
        pre_w[("h", 0)] = wht0
        if split:
            wl_r0 = wl_d.rearrange("(k p) (o m) -> o p k m", p=128, m=128)
            wlt0 = wpool.tile([128, NK * 128], F16, tag="wl", name="wlt0")
            nc.sync.dma_start(
                wlt0[:].rearrange("p (k m) -> p k m", m=128), wl_r0[0])
            pre_w[("l", 0)] = wlt0

        # ---- resident x loads: per-k tiles [128, TB] ----
        xh_t = xh_d.rearrange("(k p) n -> k p n", p=128)
        if split:
            xl_t = xl_d.rearrange("(k p) n -> k p n", p=128)
        x_hi, x_lo = [], []
        for k in range(NK):
            xt = xres.tile([128, TB], mm_dt, tag="xh", name=f"xh{k}")
            nc.scalar.dma_start(xt[:], xh_t[k])
            x_hi.append(xt)
            if split:
                xt2 = xres.tile([128, TB], F16, tag="xl", name=f"xl{k}")
                nc.scalar.dma_start(xt2[:], xl_t[k])
                x_lo.append(xt2)

        bias_t = mpool.tile([128, NOC], F32, tag="bias")
        nc.sync.dma_start(bias_t[:], bias_d[:, :])
        auxw_t = mpool.tile([128, NOC * NCLS], F32, tag="auxw")
        nc.sync.dma_start(
            auxw_t[:].rearrange("p (j n) -> p j n", n=NCLS),
            auxwT.rearrange("(j p) n -> p j n", p=128),
        )
        auxb_t = mpool.tile([BL, NCLS], F32, tag="auxb")
        nc.sync.dma_start(auxb_t[:], auxb[:, :])

        wh_r = wh_d.rearrange("(k p) (o m) -> o p k m", p=128, m=128)
        if split:
            wl_r = wl_d.rearrange("(k p) (o m) -> o p k m", p=128, m=128)

        cur_tiles = {}   # (g, nb) -> tile [128, TPB*OCG*BL]
        cnt_tiles = {}   # g -> final count tile [128, GW]
        lg_parts = {}    # g -> SBUF partial logits [BL, NCLS]

        def emit_mm_group(g):
            for j in range(OCG):
                oc = g * OCG + j
                if ("h", oc) in pre_w:
                    wht = pre_w[("h", oc)]
                else:
                    wht = wpool.tile([128, NK * 128], mm_dt, tag="wh")
                    nc.sync.dma_start(
                        wht[:].rearrange("p (k m) -> p k m", m=128), wh_r[oc])
                if split:
                    if ("l", oc) in pre_w:
                        wlt = pre_w[("l", oc)]
                    else:
                        wlt = wpool.tile([128, NK * 128], F16, tag="wl")
                        nc.sync.dma_start(
                            wlt[:].rearrange("p (k m) -> p k m", m=128),
                            wl_r[oc])
                for nb in range(NB):
                    if (g, nb) not in cur_tiles:
                        cur_tiles[(g, nb)] = curpool.tile(
                            [128, TPB * OCG * BL], F32, tag="cur",
                            name=f"cur_{g}_{nb}")
                    cur = cur_tiles[(g, nb)]
                    csl = slice(nb * NBW, (nb + 1) * NBW)
                    psumA = ppool.tile([128, NBW], F32, tag="mm")
                    if split:
                        psumB = ppool.tile([128, NBW], F32, tag="mm")
                    for k in range(NK):
                        whk = wht[:, k * 128:(k + 1) * 128]
                        nc.tensor.matmul(psumA[:], whk, x_hi[k][:, csl],
                                         start=(k == 0), stop=(k == NK - 1))
                        if split:
                            nc.tensor.matmul(psumB[:], whk, x_lo[k][:, csl],
                                             start=(k == 0), stop=False)
                            nc.tensor.matmul(psumB[:],
                                             wlt[:, k * 128:(k + 1) * 128],
                                             x_hi[k][:, csl],
                                             start=False, stop=(k == NK - 1))
                    # psum [p, (t_local, b)] -> cur [p, (t_local, j, b)]
                    out_ap = cur[:].rearrange(
                        "p (t j b) -> p t j b", t=TPB, j=OCG, b=BL)[:, :, j, :]
                    psA = psumA[:].rearrange("p (t b) -> p t b", b=BL)
                    nc.scalar.activation(
                        out_ap, psA, mybir.ActivationFunctionType.Identity,
                        bias=bias_t[:, oc:oc + 1], scale=1.0)
                    if split:
                        psB = psumB[:].rearrange("p (t b) -> p t b", b=BL)
                        nc.vector.scalar_tensor_tensor(
                            out_ap, psB, 1.0 / LO_SCALE, out_ap,
                            op0=mybir.AluOpType.mult, op1=mybir.AluOpType.add)

        def emit_scan_group(g):
            v = vpool.tile([128, GW], F32, tag=f"v{g}")
            nc.vector.memset(v[:], 0.0)
            c_acc = cpool.tile([128, GW], F32, tag=f"c{g}")
            nc.vector.memset(c_acc[:], 0.0)
            for t in range(T):
                nb, tl = divmod(t, TPB)
                c_slice = cur_tiles[(g, nb)][:, tl * GW:(tl + 1) * GW]
                v2 = vpool.tile([128, GW], F32, tag=f"v{g}")
                nc.vector.scalar_tensor_tensor(
                    v2[:], v[:], 0.5, c_slice,
                    op0=mybir.AluOpType.mult, op1=mybir.AluOpType.add)
                s = spool.tile([128, GW], BF16, tag="s")
                nc.vector.tensor_scalar(
                    s[:], v2[:], 1.0, None, op0=mybir.AluOpType.is_ge)
                c2 = cpool.tile([128, GW], F32, tag=f"c{g}")
                nc.vector.tensor_tensor(
                    c2[:], s[:], c_acc[:], op=mybir.AluOpType.add)
                c_acc = c2
                v3 = vpool.tile([128, GW], F32, tag=f"v{g}")
                nc.vector.scalar_tensor_tensor(
                    v3[:], v2[:], 1.0, v2[:],
                    op0=mybir.AluOpType.is_lt, op1=mybir.AluOpType.mult)
                v = v3
                nc.scalar.dma_start(
                    spk[t, g * OCG:(g + 1) * OCG].rearrange("o p b -> p o b"),
                    s[:].rearrange("p (o b) -> p o b", b=BL))
            cnt_tiles[g] = c_acc
            nc.sync.dma_start(
                cnt[g * OCG:(g + 1) * OCG].rearrange("o p b -> p o b"),
                c_acc[:].rearrange("p (o b) -> p o b", b=BL))
            lgp = ppool.tile([BL, NCLS], F32, tag="lg", name=f"lgp{g}")
            for j in range(OCG):
                oc = g * OCG + j
                nc.tensor.matmul(
                    lgp[:], c_acc[:, j * BL:(j + 1) * BL],
                    auxw_t[:, oc * NCLS:(oc + 1) * NCLS],
                    start=(j == 0), stop=(j == OCG - 1))
            lg_sb = mpool.tile([BL, NCLS], F32, tag=f"lg{g}", name=f"lgsb{g}")
            nc.scalar.copy(lg_sb[:], lgp[:])
            lg_parts[g] = lg_sb

        # Each scan group computes its partial logits contribution; the
        # partials are summed sequentially at the end (same fp32 rounding
        # order as one long accumulation).
        for g in range(G):
            emit_mm_group(g)
            emit_scan_group(g)

        acc = lg_parts[0]
        for g in range(1, G):
            nxt = mpool.tile([BL, NCLS], F32, tag=f"lacc{g}", name=f"lacc{g}")
            nc.vector.tensor_tensor(nxt[:], acc[:], lg_parts[g][:],
                                    op=mybir.AluOpType.add)
            acc = nxt
        lsb = mpool.tile([BL, NCLS], F32, tag="lgs")
        nc.vector.tensor_tensor(lsb[:], acc[:], auxb_t[:],
                                op=mybir.AluOpType.add)
        nc.sync.dma_start(logits, lsb[:])

    nc.compile()
    return nc


_NC = None


def kernel(x_seq, fc_w, fc_b, aux_w, aux_b):
    global _NC, LAST_RESULTS
    if _NC is None:
        _NC = _build_nc()

    split = MM_MODE == "fp16x3"
    wT32 = fc_w.astype(np.float32).T * np.float32(0.5)   # [IN, OUT]
    bias = np.ascontiguousarray(
        (fc_b.astype(np.float32) * np.float32(0.5)).reshape(NOC, 128).T)
    auxwT = np.ascontiguousarray(aux_w.astype(np.float32).T)
    auxb = np.ascontiguousarray(
        np.broadcast_to(aux_b.astype(np.float32), (BL, NCLS)))

    if split:
        wh = wT32.astype(np.float16)
        wl = ((wT32 - wh.astype(np.float32)) * np.float32(LO_SCALE)
              ).astype(np.float16)
        wh = np.ascontiguousarray(wh)
        wl = np.ascontiguousarray(wl)
    else:
        wh = np.ascontiguousarray(wT32)

    in_maps = []
    for c in range(NCORES):
        xs = x_seq[:, c * BL:(c + 1) * BL, :].astype(np.float32)
        xT = np.ascontiguousarray(xs.transpose(2, 0, 1).reshape(IN, TB))
        m = {"bias": bias, "auxwT": auxwT, "auxb": auxb, "wh": wh}
        if split:
            xh = xT.astype(np.float16)
            xl = ((xT - xh.astype(np.float32)) * np.float32(LO_SCALE)
                  ).astype(np.float16)
            m["xh"] = np.ascontiguousarray(xh)
            m["xl"] = np.ascontiguousarray(xl)
            m["wl"] = wl
        else:
            m["xh"] = xT
        in_maps.append(m)

    res = run_bass_kernel_spmd(
        _NC, in_maps, core_ids=list(range(NCORES)), trace=TRACE)
    LAST_RESULTS = res

    spk_parts, cnt_parts, log_parts = [], [], []
    for c in range(NCORES):
        r = res.results[c]
        spk_c = np.asarray(r["spk"]).astype(np.float32)
        spk_parts.append(spk_c.transpose(0, 3, 1, 2).reshape(T, BL, OUT))
        cnt_parts.append(
            np.asarray(r["cnt"]).transpose(2, 0, 1).reshape(BL, OUT))
        log_parts.append(np.asarray(r["logits"]))
    spk_seq = np.concatenate(spk_parts, axis=1)
    count = np.concatenate(cnt_parts, axis=0)
    logits = np.concatenate(log_parts, axis=0)
    return spk_seq, count, logits


# revision 19
# speedup vs baseline: 1.1579x; 1.1579x over previous
"""Trainium2 Bass kernel for nn_LocalGreedyLayer (LIF spiking layer).

Computes, for x_seq [T=16, B=512, IN=3072], fc_w [2048,3072], fc_b [2048],
aux_w [10,2048], aux_b [10]:
    cur  = x_seq @ fc_w.T + fc_b            # [T,B,OUT]
    LIF scan (tau=2, v_th=1, hard reset to 0) -> spk_seq [T,B,OUT]
    count = spk_seq.sum(0)                  # [B,OUT]
    logits = count @ aux_w.T + aux_b        # [B,10]

Sharding: data-parallel over batch B across 8 NeuronCores (64 batch rows
per core); weights replicated. Per core the matmul is computed in a
transposed layout curT [OUT, T*64] so OUT lives on SBUF partitions, the
time scan runs on [128, free] tiles (partition = out%128, free =
(out//128, b)), and spikes/counts are emitted transposed; the host
reassembles the full outputs.

Matmul modes:
  fp32   - native fp32 matmuls (4 cyc/row), bit-accurate.
  fp32r  - single-pass reduced-precision fp32 (~13 mantissa bits, 1 cyc/row).
  fp16x3 - hi/lo fp16 split: cur = xh@wh + 2^-11*(xh@wl' + xl'@wh) with
           lo parts pre-scaled by 2^11 on the host. ~22 effective mantissa
           bits (empirically fp32-equivalent spike decisions) at 3 cyc/row.
"""

import sys

sys.path.insert(0, "/opt/trn_rl_repo")

from contextlib import ExitStack

import numpy as np

import concourse.bass as bass  # noqa: F401
import concourse.tile as tile
from concourse import bacc, mybir
from concourse.bass_utils import run_bass_kernel_spmd

T, B, IN, OUT, NCLS = 16, 512, 3072, 2048, 10
NCORES = 8
BL = B // NCORES          # 64 batch rows per core
TB = T * BL               # 1024 columns of curT per core
NK = IN // 128            # 24 contraction chunks
NOC = OUT // 128          # 16 output chunks
NB = 2                    # N-blocks of 512 columns (8 timesteps each)
NBW = TB // NB            # 512
TPB = T // NB             # 8 timesteps per N-block
G = 2                     # oc groups (scan overlap granularity)
OCG = NOC // G            # 8 oc per group
GW = OCG * BL             # 512 free elements per scan-group row

F32 = mybir.dt.float32
F16 = mybir.dt.float16
BF16 = mybir.dt.bfloat16

LO_SCALE = 2048.0         # 2^11 scaling of fp16 lo parts

MM_MODE = "fp16x3"

TRACE = False
LAST_RESULTS = None


def _build_nc():
    nc = bacc.Bacc("TRN2", target_bir_lowering=False, debug=False,
                   enable_asserts=False, num_devices=NCORES)

    split = MM_MODE == "fp16x3"
    mm_dt = {"fp32": F32, "fp32r": mybir.dt.float32r, "fp16x3": F16}[MM_MODE]

    if split:
        xh_d = nc.dram_tensor("xh", [IN, TB], F16, kind="ExternalInput").ap()
        xl_d = nc.dram_tensor("xl", [IN, TB], F16, kind="ExternalInput").ap()
        wh_d = nc.dram_tensor("wh", [IN, OUT], F16, kind="ExternalInput").ap()
        wl_d = nc.dram_tensor("wl", [IN, OUT], F16, kind="ExternalInput").ap()
    else:
        xh_d = nc.dram_tensor("xh", [IN, TB], mm_dt, kind="ExternalInput").ap()
        wh_d = nc.dram_tensor("wh", [IN, OUT], mm_dt, kind="ExternalInput").ap()
    bias_d = nc.dram_tensor("bias", [128, NOC], F32, kind="ExternalInput").ap()
    auxwT = nc.dram_tensor("auxwT", [OUT, NCLS], F32, kind="ExternalInput").ap()
    auxb = nc.dram_tensor("auxb", [BL, NCLS], F32, kind="ExternalInput").ap()

    spk = nc.dram_tensor("spk", [T, NOC, 128, BL], BF16, kind="ExternalOutput").ap()
    cnt = nc.dram_tensor("cnt", [NOC, 128, BL], F32, kind="ExternalOutput").ap()
    logits = nc.dram_tensor("logits", [BL, NCLS], F32, kind="ExternalOutput").ap()

    with tile.TileContext(nc) as tc, ExitStack() as ctx:
        xres = ctx.enter_context(tc.tile_pool(name="xres", bufs=NK))
        wpool = ctx.enter_context(tc.tile_pool(name="w", bufs=2))
        curpool = ctx.enter_context(tc.tile_pool(name="cur", bufs=4))
        vpool = ctx.enter_context(tc.tile_pool(name="v", bufs=2))
        spool = ctx.enter_context(tc.tile_pool(name="s", bufs=3))
        cpool = ctx.enter_context(tc.tile_pool(name="cnt", bufs=2))
        mpool = ctx.enter_context(tc.tile_pool(name="misc", bufs=1))
        ppool = ctx.enter_context(tc.tile_pool(name="psum", bufs=4, space="PSUM"))


        wh_r0 = wh_d.rearrange("(k p) (o m) -> o p k m", p=128, m=128)
        # Preload the first oc's weight tiles ahead of the x stream so the
        # PE can start as soon as x[k=0] lands. w loads go on the sync
        # engine's DGE queues; x loads/spk stores on the scalar engine's,
        # so the two streams don't serialize behind each other.
        pre_w = {}
        wht0 = wpool.tile([128, NK * 128], mm_dt, tag="wh", name="wht0")
        nc.sync.dma_start(
            wht0[:].rearrange("p (k m) -> p k m", m=128), wh_r0[0])
        pre_w[("h", 0)] = wht0
        if split:
            wl_r0 = wl_d.rearrange("(k p) (o m) -> o p k m", p=128, m=128)
            wlt0 = wpool.tile([128, NK * 128], F16, tag="wl", name="wlt0")
            nc.sync.dma_start(
                wlt0[:].rearrange("p (k m) -> p k m", m=128), wl_r0[0])
            pre_w[("l", 0)] = wlt0

        # ---- resident x loads: per-k tiles [128, TB] ----
        xh_t = xh_d.rearrange("(k p) n -> k p n", p=128)
        if split:
            xl_t = xl_d.rearrange("(k p) n -> k p n", p=128)
        x_hi, x_lo = [], []
        for k in range(NK):
            xt = xres.tile([128, TB], mm_dt, tag="xh", name=f"xh{k}")
            nc.scalar.dma_start(xt[:], xh_t[k])
            x_hi.append(xt)
            if split:
                xt2 = xres.tile([128, TB], F16, tag="xl", name=f"xl{k}")
                nc.scalar.dma_start(xt2[:], xl_t[k])
                x_lo.append(xt2)

        bias_t = mpool.tile([128, NOC], F32, tag="bias")
        nc.sync.dma_start(bias_t[:], bias_d[:, :])
        auxw_t = mpool.tile([128, NOC * NCLS], F32, tag="auxw")
        nc.sync.dma_start(
            auxw_t[:].rearrange("p (j n) -> p j n", n=NCLS),
            auxwT.rearrange("(j p) n -> p j n", p=128),
        )
        auxb_t = mpool.tile([BL, NCLS], F32, tag="auxb")
        nc.sync.dma_start(auxb_t[:], auxb[:, :])

        wh_r = wh_d.rearrange("(k p) (o m) -> o p k m", p=128, m=128)
        if split:
            wl_r = wl_d.rearrange("(k p) (o m) -> o p k m", p=128, m=128)

        cur_tiles = {}   # (g, nb) -> tile [128, TPB*OCG*BL]
        cnt_tiles = {}   # g -> final count tile [128, GW]
        lg_parts = {}    # g -> SBUF partial logits [BL, NCLS]

        def emit_mm_group(g):
            for j in range(OCG):
                oc = g * OCG + j
                if ("h", oc) in pre_w:
                    wht = pre_w[("h", oc)]
                else:
                    wht = wpool.tile([128, NK * 128], mm_dt, tag="wh")
                    nc.sync.dma_start(
                        wht[:].rearrange("p (k m) -> p k m", m=128), wh_r[oc])
                if split:
                    if ("l", oc) in pre_w:
                        wlt = pre_w[("l", oc)]
                    else:
                        wlt = wpool.tile([128, NK * 128], F16, tag="wl")
                        nc.sync.dma_start(
                            wlt[:].rearrange("p (k m) -> p k m", m=128),
                            wl_r[oc])
                for nb in range(NB):
                    if (g, nb) not in cur_tiles:
                        cur_tiles[(g, nb)] = curpool.tile(
                            [128, TPB * OCG * BL], F32, tag="cur",
                            name=f"cur_{g}_{nb}")
                    cur = cur_tiles[(g, nb)]
                    csl = slice(nb * NBW, (nb + 1) * NBW)
                    psumA = ppool.tile([128, NBW], F32, tag="mm")
                    if split:
                        psumB = ppool.tile([128, NBW], F32, tag="mm")
                    for k in range(NK):
                        whk = wht[:, k * 128:(k + 1) * 128]
                        nc.tensor.matmul(psumA[:], whk, x_hi[k][:, csl],
                                         start=(k == 0), stop=(k == NK - 1))
                        if split:
                            nc.tensor.matmul(psumB[:], whk, x_lo[k][:, csl],
                                             start=(k == 0), stop=False)
                            nc.tensor.matmul(psumB[:],
                                             wlt[:, k * 128:(k + 1) * 128],
                                             x_hi[k][:, csl],
                                             start=False, stop=(k == NK - 1))
                    # psum [p, (t_local, b)] -> cur [p, (t_local, j, b)]
                    out_ap = cur[:].rearrange(
                        "p (t j b) -> p t j b", t=TPB, j=OCG, b=BL)[:, :, j, :]
                    psA = psumA[:].rearrange("p (t b) -> p t b", b=BL)
                    nc.scalar.activation(
                        out_ap, psA, mybir.ActivationFunctionType.Identity,
                        bias=bias_t[:, oc:oc + 1], scale=1.0)
                    if split:
                        psB = psumB[:].rearrange("p (t b) -> p t b", b=BL)
                        nc.vector.scalar_tensor_tensor(
                            out_ap, psB, 1.0 / LO_SCALE, out_ap,
                            op0=mybir.AluOpType.mult, op1=mybir.AluOpType.add)

        def emit_scan_group(g):
            v = vpool.tile([128, GW], F32, tag=f"v{g}")
            nc.vector.memset(v[:], 0.0)
            c_acc = cpool.tile([128, GW], F32, tag=f"c{g}")
            nc.vector.memset(c_acc[:], 0.0)
            for t in range(T):
                nb, tl = divmod(t, TPB)
                c_slice = cur_tiles[(g, nb)][:, tl * GW:(tl + 1) * GW]
                v2 = vpool.tile([128, GW], F32, tag=f"v{g}")
                nc.vector.scalar_tensor_tensor(
                    v2[:], v[:], 0.5, c_slice,
                    op0=mybir.AluOpType.mult, op1=mybir.AluOpType.add)
                s = spool.tile([128, GW], BF16, tag="s")
                nc.vector.tensor_scalar(
                    s[:], v2[:], 1.0, None, op0=mybir.AluOpType.is_ge)
                c2 = cpool.tile([128, GW], F32, tag=f"c{g}")
                nc.vector.tensor_tensor(
                    c2[:], s[:], c_acc[:], op=mybir.AluOpType.add)
                c_acc = c2
                v3 = vpool.tile([128, GW], F32, tag=f"v{g}")
                nc.vector.scalar_tensor_tensor(
                    v3[:], v2[:], 1.0, v2[:],
                    op0=mybir.AluOpType.is_lt, op1=mybir.AluOpType.mult)
                v = v3
                nc.scalar.dma_start(
                    spk[t, g * OCG:(g + 1) * OCG].rearrange("o p b -> p o b"),
                    s[:].rearrange("p (o b) -> p o b", b=BL))
            cnt_tiles[g] = c_acc
            nc.sync.dma_start(
                cnt[g * OCG:(g + 1) * OCG].rearrange("o p b -> p o b"),
                c_acc[:].rearrange("p (o b) -> p o b", b=BL))
            lgp = ppool.tile([BL, NCLS], F32, tag="lg", name=f"lgp{g}")
            for j in range(OCG):
                oc = g * OCG + j
                nc.tensor.matmul(
                    lgp[:], c_acc[:, j * BL:(j + 1) * BL],
                    auxw_t[:, oc * NCLS:(oc + 1) * NCLS],
                    start=(j == 0), stop=(j == OCG - 1))
            lg_sb = mpool.tile([BL, NCLS], F32, tag=f"lg{g}", name=f"lgsb{g}")
            nc.scalar.copy(lg_sb[:], lgp[:])
            lg_parts[g] = lg_sb

        # Each scan group computes its partial logits contribution; the
        # partials are summed sequentially at the end (same fp32 rounding
        # order as one long accumulation).
        for g in range(G):
            emit_mm_group(g)
            emit_scan_group(g)

        acc = lg_parts[0]
        for g in range(1, G):
            nxt = mpool.tile([BL, NCLS], F32, tag=f"lacc{g}", name=f"lacc{g}")
            nc.vector.tensor_tensor(nxt[:], acc[:], lg_parts[g][:],
                                    op=mybir.AluOpType.add)
            acc = nxt
        lsb = mpool.tile([BL, NCLS], F32, tag="lgs")
        nc.vector.tensor_tensor(lsb[:], acc[:], auxb_t[:],
                                op=mybir.AluOpType.add)
        nc.sync.dma_start(logits, lsb[:])

    nc.compile()
    return nc


_NC = None


def kernel(x_seq, fc_w, fc_b, aux_w, aux_b):
    global _NC, LAST_RESULTS
    if _NC is None:
        _NC = _build_nc()

    split = MM_MODE == "fp16x3"
    wT32 = fc_w.astype(np.float32).T * np.float32(0.5)   # [IN, OUT]
    bias = np.ascontiguousarray(
        (fc_b.astype(np.float32) * np.float32(0.5)).reshape(NOC, 128).T)
    auxwT = np.ascontiguousarray(aux_w.astype(np.float32).T)
    auxb = np.ascontiguousarray(
        np.broadcast_to(aux_b.astype(np.float32), (BL, NCLS)))

    if split:
        wh = wT32.astype(np.float16)
        wl = ((wT32 - wh.astype(np.float32)) * np.float32(LO_SCALE)
              ).astype(np.float16)
        wh = np.ascontiguousarray(wh)
        wl = np.ascontiguousarray(wl)
    else:
        wh = np.ascontiguousarray(wT32)

    in_maps = []
    for c in range(NCORES):
        xs = x_seq[:, c * BL:(c + 1) * BL, :].astype(np.float32)
        xT = np.ascontiguousarray(xs.transpose(2, 0, 1).reshape(IN, TB))
        m = {"bias": bias, "auxwT": auxwT, "auxb": auxb, "wh": wh}
        if split:
            xh = xT.astype(np.float16)
            xl = ((xT - xh.astype(np.float32)) * np.float32(LO_SCALE)
                  ).astype(np.float16)
            m["xh"] = np.ascontiguousarray(xh)
            m["xl"] = np.ascontiguousarray(xl)
            m["wl"] = wl
        else:
            m["xh"] = xT
        in_maps.append(m)

    res = run_bass_kernel_spmd(
        _NC, in_maps, core_ids=list(range(NCORES)), trace=TRACE)
    LAST_RESULTS = res

    spk_parts, cnt_parts, log_parts = [], [], []
    for c in range(NCORES):
        r = res.results[c]
        spk_c = np.asarray(r["spk"]).astype(np.float32)
        spk_parts.append(spk_c.transpose(0, 3, 1, 2).reshape(T, BL, OUT))
        cnt_parts.append(
            np.asarray(r["cnt"]).transpose(2, 0, 1).reshape(BL, OUT))
        log_parts.append(np.asarray(r["logits"]))
    spk_seq = np.concatenate(spk_parts, axis=1)
    count = np.concatenate(cnt_parts, axis=0)
    logits = np.concatenate(log_parts, axis=0)
    return spk_seq, count, logits


# revision 20
# speedup vs baseline: 2.4550x; 2.1202x over previous
"""Trainium2 Bass kernel for nn_LocalGreedyLayer (LIF spiking layer).

Computes, for x_seq [T=16, B=512, IN=3072], fc_w [2048,3072], fc_b [2048],
aux_w [10,2048], aux_b [10]:
    cur  = x_seq @ fc_w.T + fc_b            # [T,B,OUT]
    LIF scan (tau=2, v_th=1, hard reset to 0) -> spk_seq [T,B,OUT]
    count = spk_seq.sum(0)                  # [B,OUT]
    logits = count @ aux_w.T + aux_b        # [B,10]

Sharding: data-parallel over batch B across 8 NeuronCores (64 batch rows
per core); weights replicated. Per core the matmul is computed in a
transposed layout curT [OUT, T*64] so OUT lives on SBUF partitions, the
time scan runs on [128, free] tiles (partition = out%128, free =
(out//128, b)), and spikes/counts are emitted transposed; the host
reassembles the full outputs.

Matmul modes:
  fp32   - native fp32 matmuls (4 cyc/row), bit-accurate.
  fp32r  - single-pass reduced-precision fp32 (~13 mantissa bits, 1 cyc/row).
  fp16x3 - hi/lo fp16 split: cur = xh@wh + 2^-11*(xh@wl' + xl'@wh) with
           lo parts pre-scaled by 2^11 on the host. ~22 effective mantissa
           bits (empirically fp32-equivalent spike decisions) at 3 cyc/row.
"""

import sys

sys.path.insert(0, "/opt/trn_rl_repo")

from contextlib import ExitStack

import numpy as np

import concourse.bass as bass  # noqa: F401
import concourse.tile as tile
from concourse import bacc, mybir
from concourse.bass_utils import run_bass_kernel_spmd

T, B, IN, OUT, NCLS = 16, 512, 3072, 2048, 10
NCORES = 8
BL = B // NCORES          # 64 batch rows per core
TB = T * BL               # 1024 columns of curT per core
NK = IN // 128            # 24 contraction chunks
NOC = OUT // 128          # 16 output chunks
NB = 2                    # N-blocks of 512 columns (8 timesteps each)
NBW = TB // NB            # 512
TPB = T // NB             # 8 timesteps per N-block
G = 2                     # oc groups (scan overlap granularity)
OCG = NOC // G            # 8 oc per group
GW = OCG * BL             # 512 free elements per scan-group row

F32 = mybir.dt.float32
F16 = mybir.dt.float16
BF16 = mybir.dt.bfloat16

LO_SCALE = 2048.0         # 2^11 scaling of fp16 lo parts

MM_MODE = "fp32r"

TRACE = False
LAST_RESULTS = None


def _build_nc():
    nc = bacc.Bacc("TRN2", target_bir_lowering=False, debug=False,
                   enable_asserts=False, num_devices=NCORES)

    split = MM_MODE == "fp16x3"
    mm_dt = {"fp32": F32, "fp32r": mybir.dt.float32r, "fp16x3": F16}[MM_MODE]

    if split:
        xh_d = nc.dram_tensor("xh", [IN, TB], F16, kind="ExternalInput").ap()
        xl_d = nc.dram_tensor("xl", [IN, TB], F16, kind="ExternalInput").ap()
        wh_d = nc.dram_tensor("wh", [IN, OUT], F16, kind="ExternalInput").ap()
        wl_d = nc.dram_tensor("wl", [IN, OUT], F16, kind="ExternalInput").ap()
    else:
        xh_d = nc.dram_tensor("xh", [IN, TB], mm_dt, kind="ExternalInput").ap()
        wh_d = nc.dram_tensor("wh", [IN, OUT], mm_dt, kind="ExternalInput").ap()
    bias_d = nc.dram_tensor("bias", [128, NOC], F32, kind="ExternalInput").ap()
    auxwT = nc.dram_tensor("auxwT", [OUT, NCLS], F32, kind="ExternalInput").ap()
    auxb = nc.dram_tensor("auxb", [BL, NCLS], F32, kind="ExternalInput").ap()

    spk = nc.dram_tensor("spk", [T, NOC, 128, BL], BF16, kind="ExternalOutput").ap()
    cnt = nc.dram_tensor("cnt", [NOC, 128, BL], F32, kind="ExternalOutput").ap()
    logits = nc.dram_tensor("logits", [BL, NCLS], F32, kind="ExternalOutput").ap()

    with tile.TileContext(nc) as tc, ExitStack() as ctx:
        xres = ctx.enter_context(tc.tile_pool(name="xres", bufs=NK))
        wpool = ctx.enter_context(tc.tile_pool(name="w", bufs=2))
        curpool = ctx.enter_context(tc.tile_pool(name="cur", bufs=4))
        vpool = ctx.enter_context(tc.tile_pool(name="v", bufs=2))
        spool = ctx.enter_context(tc.tile_pool(name="s", bufs=3))
        cpool = ctx.enter_context(tc.tile_pool(name="cnt", bufs=2))
        mpool = ctx.enter_context(tc.tile_pool(name="misc", bufs=1))
        ppool = ctx.enter_context(tc.tile_pool(name="psum", bufs=4, space="PSUM"))


        wh_r0 = wh_d.rearrange("(k p) (o m) -> o p k m", p=128, m=128)
        # Preload the first oc's weight tiles ahead of the x stream so the
        # PE can start as soon as x[k=0] lands. w loads go on the sync
        # engine's DGE queues; x loads/spk stores on the scalar engine's,
        # so the two streams don't serialize behind each other.
        pre_w = {}
        wht0 = wpool.tile([128, NK * 128], mm_dt, tag="wh", name="wht0")
        nc.sync.dma_start(
            wht0[:].rearrange("p (k m) -> p k m", m=128), wh_r0[0])
        pre_w[("h", 0)] = wht0
        if split:
            wl_r0 = wl_d.rearrange("(k p) (o m) -> o p k m", p=128, m=128)
            wlt0 = wpool.tile([128, NK * 128], F16, tag="wl", name="wlt0")
            nc.sync.dma_start(
                wlt0[:].rearrange("p (k m) -> p k m", m=128), wl_r0[0])
            pre_w[("l", 0)] = wlt0

        # ---- resident x loads: per-k tiles [128, TB] ----
        xh_t = xh_d.rearrange("(k p) n -> k p n", p=128)
        if split:
            xl_t = xl_d.rearrange("(k p) n -> k p n", p=128)
        x_hi, x_lo = [], []
        for k in range(NK):
            xt = xres.tile([128, TB], mm_dt, tag="xh", name=f"xh{k}")
            nc.scalar.dma_start(xt[:], xh_t[k])
            x_hi.append(xt)
            if split:
                xt2 = xres.tile([128, TB], F16, tag="xl", name=f"xl{k}")
                nc.scalar.dma_start(xt2[:], xl_t[k])
                x_lo.append(xt2)

        bias_t = mpool.tile([128, NOC], F32, tag="bias")
        nc.sync.dma_start(bias_t[:], bias_d[:, :])
        auxw_t = mpool.tile([128, NOC * NCLS], F32, tag="auxw")
        nc.sync.dma_start(
            auxw_t[:].rearrange("p (j n) -> p j n", n=NCLS),
            auxwT.rearrange("(j p) n -> p j n", p=128),
        )
        auxb_t = mpool.tile([BL, NCLS], F32, tag="auxb")
        nc.sync.dma_start(auxb_t[:], auxb[:, :])

        wh_r = wh_d.rearrange("(k p) (o m) -> o p k m", p=128, m=128)
        if split:
            wl_r = wl_d.rearrange("(k p) (o m) -> o p k m", p=128, m=128)

        cur_tiles = {}   # (g, nb) -> tile [128, TPB*OCG*BL]
        cnt_tiles = {}   # g -> final count tile [128, GW]
        lg_parts = {}    # g -> SBUF partial logits [BL, NCLS]

        def emit_mm_group(g):
            for j in range(OCG):
                oc = g * OCG + j
                if ("h", oc) in pre_w:
                    wht = pre_w[("h", oc)]
                else:
                    wht = wpool.tile([128, NK * 128], mm_dt, tag="wh")
                    nc.sync.dma_start(
                        wht[:].rearrange("p (k m) -> p k m", m=128), wh_r[oc])
                if split:
                    if ("l", oc) in pre_w:
                        wlt = pre_w[("l", oc)]
                    else:
                        wlt = wpool.tile([128, NK * 128], F16, tag="wl")
                        nc.sync.dma_start(
                            wlt[:].rearrange("p (k m) -> p k m", m=128),
                            wl_r[oc])
                for nb in range(NB):
                    if (g, nb) not in cur_tiles:
                        cur_tiles[(g, nb)] = curpool.tile(
                            [128, TPB * OCG * BL], F32, tag="cur",
                            name=f"cur_{g}_{nb}")
                    cur = cur_tiles[(g, nb)]
                    csl = slice(nb * NBW, (nb + 1) * NBW)
                    psumA = ppool.tile([128, NBW], F32, tag="mm")
                    if split:
                        psumB = ppool.tile([128, NBW], F32, tag="mm")
                    for k in range(NK):
                        whk = wht[:, k * 128:(k + 1) * 128]
                        nc.tensor.matmul(psumA[:], whk, x_hi[k][:, csl],
                                         start=(k == 0), stop=(k == NK - 1))
                        if split:
                            nc.tensor.matmul(psumB[:], whk, x_lo[k][:, csl],
                                             start=(k == 0), stop=False)
                            nc.tensor.matmul(psumB[:],
                                             wlt[:, k * 128:(k + 1) * 128],
                                             x_hi[k][:, csl],
                                             start=False, stop=(k == NK - 1))
                    # psum [p, (t_local, b)] -> cur [p, (t_local, j, b)]
                    out_ap = cur[:].rearrange(
                        "p (t j b) -> p t j b", t=TPB, j=OCG, b=BL)[:, :, j, :]
                    psA = psumA[:].rearrange("p (t b) -> p t b", b=BL)
                    nc.scalar.activation(
                        out_ap, psA, mybir.ActivationFunctionType.Identity,
                        bias=bias_t[:, oc:oc + 1], scale=1.0)
                    if split:
                        psB = psumB[:].rearrange("p (t b) -> p t b", b=BL)
                        nc.vector.scalar_tensor_tensor(
                            out_ap, psB, 1.0 / LO_SCALE, out_ap,
                            op0=mybir.AluOpType.mult, op1=mybir.AluOpType.add)

        def emit_scan_group(g):
            v = vpool.tile([128, GW], F32, tag=f"v{g}")
            nc.vector.memset(v[:], 0.0)
            c_acc = cpool.tile([128, GW], F32, tag=f"c{g}")
            nc.vector.memset(c_acc[:], 0.0)
            for t in range(T):
                nb, tl = divmod(t, TPB)
                c_slice = cur_tiles[(g, nb)][:, tl * GW:(tl + 1) * GW]
                v2 = vpool.tile([128, GW], F32, tag=f"v{g}")
                nc.vector.scalar_tensor_tensor(
                    v2[:], v[:], 0.5, c_slice,
                    op0=mybir.AluOpType.mult, op1=mybir.AluOpType.add)
                s = spool.tile([128, GW], BF16, tag="s")
                nc.vector.tensor_scalar(
                    s[:], v2[:], 1.0, None, op0=mybir.AluOpType.is_ge)
                c2 = cpool.tile([128, GW], F32, tag=f"c{g}")
                nc.vector.tensor_tensor(
                    c2[:], s[:], c_acc[:], op=mybir.AluOpType.add)
                c_acc = c2
                v3 = vpool.tile([128, GW], F32, tag=f"v{g}")
                nc.vector.scalar_tensor_tensor(
                    v3[:], v2[:], 1.0, v2[:],
                    op0=mybir.AluOpType.is_lt, op1=mybir.AluOpType.mult)
                v = v3
                nc.scalar.dma_start(
                    spk[t, g * OCG:(g + 1) * OCG].rearrange("o p b -> p o b"),
                    s[:].rearrange("p (o b) -> p o b", b=BL))
            cnt_tiles[g] = c_acc
            nc.sync.dma_start(
                cnt[g * OCG:(g + 1) * OCG].rearrange("o p b -> p o b"),
                c_acc[:].rearrange("p (o b) -> p o b", b=BL))
            lgp = ppool.tile([BL, NCLS], F32, tag="lg", name=f"lgp{g}")
            for j in range(OCG):
                oc = g * OCG + j
                nc.tensor.matmul(
                    lgp[:], c_acc[:, j * BL:(j + 1) * BL],
                    auxw_t[:, oc * NCLS:(oc + 1) * NCLS],
                    start=(j == 0), stop=(j == OCG - 1))
            lg_sb = mpool.tile([BL, NCLS], F32, tag=f"lg{g}", name=f"lgsb{g}")
            nc.scalar.copy(lg_sb[:], lgp[:])
            lg_parts[g] = lg_sb

        # Each scan group computes its partial logits contribution; the
        # partials are summed sequentially at the end (same fp32 rounding
        # order as one long accumulation).
        for g in range(G):
            emit_mm_group(g)
            emit_scan_group(g)

        acc = lg_parts[0]
        for g in range(1, G):
            nxt = mpool.tile([BL, NCLS], F32, tag=f"lacc{g}", name=f"lacc{g}")
            nc.vector.tensor_tensor(nxt[:], acc[:], lg_parts[g][:],
                                    op=mybir.AluOpType.add)
            acc = nxt
        lsb = mpool.tile([BL, NCLS], F32, tag="lgs")
        nc.vector.tensor_tensor(lsb[:], acc[:], auxb_t[:],
                                op=mybir.AluOpType.add)
        nc.sync.dma_start(logits, lsb[:])

    nc.compile()
    return nc


_NC = None


def kernel(x_seq, fc_w, fc_b, aux_w, aux_b):
    global _NC, LAST_RESULTS
    if _NC is None:
        _NC = _build_nc()

    split = MM_MODE == "fp16x3"
    wT32 = fc_w.astype(np.float32).T * np.float32(0.5)   # [IN, OUT]
    bias = np.ascontiguousarray(
        (fc_b.astype(np.float32) * np.float32(0.5)).reshape(NOC, 128).T)
    auxwT = np.ascontiguousarray(aux_w.astype(np.float32).T)
    auxb = np.ascontiguousarray(
        np.broadcast_to(aux_b.astype(np.float32), (BL, NCLS)))

    if split:
        wh = wT32.astype(np.float16)
        wl = ((wT32 - wh.astype(np.float32)) * np.float32(LO_SCALE)
              ).astype(np.float16)
        wh = np.ascontiguousarray(wh)
        wl = np.ascontiguousarray(wl)
    else:
        wh = np.ascontiguousarray(wT32)

    in_maps = []
    for c in range(NCORES):
        xs = x_seq[:, c * BL:(c + 1) * BL, :].astype(np.float32)
        xT = np.ascontiguousarray(xs.transpose(2, 0, 1).reshape(IN, TB))
        m = {"bias": bias, "auxwT": auxwT, "auxb": auxb, "wh": wh}
        if split:
            xh = xT.astype(np.float16)
            xl = ((xT - xh.astype(np.float32)) * np.float32(LO_SCALE)
                  ).astype(np.float16)
            m["xh"] = np.ascontiguousarray(xh)
            m["xl"] = np.ascontiguousarray(xl)
            m["wl"] = wl
        else:
            m["xh"] = xT
        in_maps.append(m)

    res = run_bass_kernel_spmd(
        _NC, in_maps, core_ids=list(range(NCORES)), trace=TRACE)
    LAST_RESULTS = res

    spk_parts, cnt_parts, log_parts = [], [], []
    for c in range(NCORES):
        r = res.results[c]
        spk_c = np.asarray(r["spk"]).astype(np.float32)
        spk_parts.append(spk_c.transpose(0, 3, 1, 2).reshape(T, BL, OUT))
        cnt_parts.append(
            np.asarray(r["cnt"]).transpose(2, 0, 1).reshape(BL, OUT))
        log_parts.append(np.asarray(r["logits"]))
    spk_seq = np.concatenate(spk_parts, axis=1)
    count = np.concatenate(cnt_parts, axis=0)
    logits = np.concatenate(log_parts, axis=0)
    return spk_seq, count, logits
